# revision 21
# baseline (speedup 1.0000x reference)
"""Trainium2 Bass kernel for GQA attention (RoPE + causal) + output projection.

Sharding: (batch, head-half) across 8 cores. Core c handles batch c//2 and
q-heads [8*(c%2), 8*(c%2)+8) with kv-heads {2*(c%2), 2*(c%2)+1}. Each core
writes two transposed partial outputs [2, D, S] in bf16 (one per kv-group);
the host sums the four partials per batch and transposes back.

Engine plan (per core):
- PE: projections (bf16), rope pair-swap, scores/AV (bf16), softmax-denominator
  column sums + broadcasts, output projection (bf16). Emission is software-
  pipelined (drains overlapped into following matmul streams, paired-score
  lookahead, deferred softmax tails, group-0 output projection interleaved
  into group-1's attention) to keep the PE stream dense so it holds the
  2.4 GHz p-state.
- Scalar (Act): rope PSUM->SBUF copies, exp on score PAIRS ([128,1024] across
  two PSUM banks, halving instruction overhead), vn copies, half the P3 copies.
- DVE: rope swap-mult, causal mask mult (bf16 2x), softmax partial-sum adds
  (bf16 2x), reciprocal_approx_fast, denominator stage copy, final normalize,
  half the P3 copies.
- Pool (gpsimd): rope mult/add (SBUF-only; Pool cannot touch PSUM), bf16 casts
  of the reciprocal row.
"""

import math
from contextlib import ExitStack
from dataclasses import dataclass

import numpy as np

import concourse.bass as bass
import concourse.tile as tile
from concourse import bacc, mybir
from concourse.bass_utils import run_bass_kernel_spmd

F32 = mybir.dt.float32
F32R = mybir.dt.float32r
BF16 = mybir.dt.bfloat16
AF = mybir.ActivationFunctionType
MUL = mybir.AluOpType.mult
ADD = mybir.AluOpType.add


@dataclass(frozen=True)
class Cfg:
    B: int = 4          # batch
    S: int = 2048       # sequence length
    D: int = 2048       # model dim
    HQC: int = 8        # q-heads per core
    HD: int = 128       # head dim
    QCH: int = 512      # chunk (matmul moving free dim)

    @property
    def DT(self):
        return self.D // 128   # d-tiles

    @property
    def KT(self):
        return self.S // 128   # 128-row tiles along S

    @property
    def NQC(self):
        return self.S // self.QCH  # q-chunks

    @property
    def RB(self):
        return self.QCH // 128     # band tiles per q-chunk


def r(ap):
    """View an fp32 AP as float32r for full-rate PE matmuls."""
    return ap.bitcast(F32R)


def build_program(cfg: Cfg):
    c = cfg
    assert c.HD == 128 and c.HQC == 8 and c.RB == 4
    nc = bacc.Bacc("TRN2", target_bir_lowering=False, debug=False)

    xt_d = nc.dram_tensor("xt", [c.D, c.S], BF16, kind="ExternalInput")
    wq_d = nc.dram_tensor("wq", [2, c.DT, 128, 4 * c.HD], BF16, kind="ExternalInput")
    wk_d = nc.dram_tensor("wk", [c.DT, 128, 2 * c.HD], BF16, kind="ExternalInput")
    wv_d = nc.dram_tensor("wv", [c.DT, 128, 2 * c.HD], BF16, kind="ExternalInput")
    wo_d = nc.dram_tensor("wo", [2, c.DT, 128, 4 * c.HD], BF16, kind="ExternalInput")
    ra_d = nc.dram_tensor("ra", [c.HD, c.S], F32, kind="ExternalInput")
    rb_d = nc.dram_tensor("rb", [c.HD, c.S], F32, kind="ExternalInput")
    cm_d = nc.dram_tensor("cm", [128, c.RB, c.QCH], BF16, kind="ExternalInput")
    pm_d = nc.dram_tensor("pm", [128, 128], F32, kind="ExternalInput")
    idn_d = nc.dram_tensor("idn", [128, 128], BF16, kind="ExternalInput")
    onec_d = nc.dram_tensor("onec", [c.HD, 1], BF16, kind="ExternalInput")
    oner_d = nc.dram_tensor("oner", [1, c.HD], BF16, kind="ExternalInput")
    out_d = nc.dram_tensor("partialT", [2, c.D, c.S], BF16, kind="ExternalOutput")

    scale = 1.0 / math.sqrt(c.HD)

    with tile.TileContext(nc) as tc, ExitStack() as ctx:
        ctx.enter_context(nc.allow_low_precision("bf16 internals; tol 2e-2"))
        const = ctx.enter_context(tc.tile_pool(name="const", bufs=1))
        wp = ctx.enter_context(tc.tile_pool(name="wp", bufs=1))
        wop = ctx.enter_context(tc.tile_pool(name="wop", bufs=3))
        xp = ctx.enter_context(tc.tile_pool(name="xp", bufs=7))
        qkp = ctx.enter_context(tc.tile_pool(name="qkp", bufs=1))
        rtp = ctx.enter_context(tc.tile_pool(name="rtp", bufs=1))
        ptp = ctx.enter_context(tc.tile_pool(name="ptp", bufs=3))
        rp = ctx.enter_context(tc.tile_pool(name="rp", bufs=2))
        ocp = ctx.enter_context(tc.tile_pool(name="ocp", bufs=3))
        ps = ctx.enter_context(
            tc.tile_pool(name="ps", bufs=1, space=bass.MemorySpace.PSUM)
        )

        # PSUM tags (8 banks): stp2 [128,1024] x2 = 4 banks (score pairs, P1 Q
        # accumulators, softmax tails), ot [128,512] x2 = 2 banks (AV
        # accumulators, P1 K/V accumulators), zz [128,512] x2 = 2 banks (rope
        # pair-swap, V transposes, P3 output accumulators).
        def p_stp2(name):
            return ps.tile([128, 2 * c.QCH], F32, name=name, tag="stp2", bufs=2)

        def p_ot(name):
            return ps.tile([128, c.QCH], F32, name=name, tag="ot", bufs=2)

        def p_zz(name, shape=None, dtype=F32):
            return ps.tile(shape or [128, c.QCH], dtype, name=name, tag="zz",
                           bufs=2)

        # ---- small constants (big tables ra/rb/cm DMA'd later) ----
        ra_sb = const.tile([128, c.S], F32, name="ra_sb")
        rb_sb = const.tile([128, c.S], F32, name="rb_sb")
        cm_sb = const.tile([128, c.RB, c.QCH], BF16, name="cm_sb")
        pm_sb = const.tile([128, 128], F32R, name="pm_sb")
        nc.sync.dma_start(pm_sb[:], r(pm_d[:]))
        idn_sb = const.tile([128, 128], BF16, name="idn_sb")
        nc.sync.dma_start(idn_sb[:], idn_d[:])
        onec_sb = const.tile([128, 1], BF16, name="onec_sb")
        nc.sync.dma_start(onec_sb[:], onec_d[:])
        oner_sb = const.tile([1, 128], BF16, name="oner_sb")
        nc.sync.dma_start(oner_sb[:], oner_d[:])

        # ---- resident weights / activations ----
        wq_sb = [
            wp.tile([128, c.DT, 4 * c.HD], BF16, name=f"wq{g}", tag=f"wq{g}")
            for g in range(2)
        ]
        wk_sb = wp.tile([128, c.DT, 2 * c.HD], BF16, name="wk_sb")
        wv_sb = wp.tile([128, c.DT, 2 * c.HD], BF16, name="wv_sb")

        q_bf = [
            qkp.tile([128, c.S], BF16, name=f"q{h}", tag=f"q{h}") for h in range(8)
        ]
        k_bf = [
            qkp.tile([128, c.S], BF16, name=f"k{g}", tag=f"k{g}") for g in range(2)
        ]
        vt = [
            qkp.tile([128, c.S], BF16, name=f"vt{g}", tag=f"vt{g}") for g in range(2)
        ]
        vn = [
            qkp.tile([128, c.KT, c.HD], BF16, name=f"vn{g}", tag=f"vn{g}")
            for g in range(2)
        ]
        ats = [
            qkp.tile([128, c.S], BF16, name=f"at{h}", tag=f"at{h}") for h in range(8)
        ]

        # ================= Phase 1: projections (+rope, V transpose) ========
        # Drains for s-chunk sc are emitted at the TOP of the next chunk's
        # loop (before the PSUM accumulators are re-allocated) so the WAR
        # dependencies are visible to the tile framework. First-read copies
        # are spread across Scalar/DVE; the pair-swap matmuls then give the
        # PE immediate work while the copies drain.
        def emit_drains(g, sc, sl, accs):
            ts = []
            for i in range(4):  # q heads: scalar/scalar/dve/dve copies
                t = rtp.tile([128, c.QCH], F32R, name=f"t{g}{sc}{i}",
                             tag="rt", bufs=3)
                if i < 2:
                    nc.scalar.copy(t[:], accs[i])
                else:
                    nc.vector.tensor_copy(t[:], accs[i])
                ts.append(t)
            tk = rtp.tile([128, c.QCH], F32R, name=f"tk{g}{sc}", tag="rt", bufs=3)
            nc.scalar.copy(tk[:], accs[4])
            ts.append(tk)
            # Pool cannot read PSUM; V drain goes on DVE (casts to bf16)
            nc.vector.tensor_copy(vt[g][:, sl], accs[5])
            # rope the 5 copied tensors (4 Q + K)
            dsts = [q_bf[g * 4 + i] for i in range(4)] + [k_bf[g]]
            for i, (t, dst) in enumerate(zip(ts, dsts)):
                rps = p_zz(f"rps{g}{sc}{i}")
                nc.tensor.matmul(rps[:], pm_sb[:], t[:])
                sw = rtp.tile([128, c.QCH], F32, name=f"sw{g}{sc}{i}",
                              tag="sw", bufs=2)
                nc.vector.tensor_tensor(sw[:], rps[:], rb_sb[:, sl], MUL)
                tr = rtp.tile([128, c.QCH], F32, name=f"tr{g}{sc}{i}",
                              tag="tr", bufs=2)
                nc.gpsimd.tensor_tensor(tr[:], t[:], ra_sb[:, sl], MUL)
                nc.gpsimd.tensor_tensor(dst[:, sl], tr[:], sw[:], ADD)

        def make_transposes(g):
            out = []
            for st_i in range(c.KT):
                def tr_one(st_i=st_i, g=g):
                    tp = p_zz(f"tp{g}{st_i}", [128, 128], BF16)
                    nc.tensor.transpose(
                        tp[:], vt[g][:, st_i * 128:(st_i + 1) * 128], idn_sb[:]
                    )
                    nc.scalar.copy(vn[g][:, st_i, :], tp[:])
                out.append(tr_one)
            return out

        drainq = []            # deferred (g, sc, sl, accs), depth 2
        pending_tr = []        # deferred V-transpose closures

        def pop_drain():
            dg, dsc, dsl, daccs = drainq.pop(0)
            emit_drains(dg, dsc, dsl, daccs)
            if dsc == c.NQC - 1:  # group dg's V is complete
                pending_tr.extend(make_transposes(dg))

        for g in range(2):
            for sc in range(c.NQC):
                sl = slice(sc * c.QCH, (sc + 1) * c.QCH)
                if len(drainq) >= 2:
                    pop_drain()
                pa = p_stp2(f"pa{g}{sc}")
                pb = p_stp2(f"pb{g}{sc}")
                accs = [pa[:, :c.QCH], pa[:, c.QCH:], pb[:, :c.QCH],
                        pb[:, c.QCH:], p_ot(f"ak{g}{sc}")[:],
                        p_ot(f"av{g}{sc}")[:]]
                for dt in range(c.DT):
                    if g == 0 and sc == 0:
                        nc.sync.dma_start(wq_sb[0][:, dt, :], wq_d[0, dt])
                        nc.sync.dma_start(wk_sb[:, dt, :], wk_d[dt])
                        nc.sync.dma_start(wv_sb[:, dt, :], wv_d[dt])
                    elif g == 0 and sc == 1:
                        nc.sync.dma_start(wq_sb[1][:, dt, :], wq_d[1, dt])
                    xt_t = xp.tile([128, c.QCH], BF16, name="xt_t", tag="xt")
                    nc.sync.dma_start(xt_t[:], xt_d[dt * 128:(dt + 1) * 128, sl])
                    st, sp = dt == 0, dt == c.DT - 1
                    for i in range(4):
                        nc.tensor.matmul(
                            accs[i],
                            wq_sb[g][:, dt, i * c.HD:(i + 1) * c.HD],
                            xt_t[:], start=st, stop=sp,
                        )
                    nc.tensor.matmul(
                        accs[4], wk_sb[:, dt, g * c.HD:(g + 1) * c.HD],
                        xt_t[:], start=st, stop=sp,
                    )
                    nc.tensor.matmul(
                        accs[5], wv_sb[:, dt, g * c.HD:(g + 1) * c.HD],
                        xt_t[:], start=st, stop=sp,
                    )
                    # big rope/mask tables ride in on the Pool DMA queue so
                    # they never contend with the SP queue's x-tile stream
                    if g == 0 and sc == 0 and dt == 8:
                        nc.gpsimd.dma_start(ra_sb[:], ra_d[:])
                        nc.gpsimd.dma_start(rb_sb[:], rb_d[:])
                        nc.gpsimd.dma_start(cm_sb[:], cm_d[:])
                    # interleave deferred V transposes of the previous group
                    if pending_tr and dt % 2 == 1:
                        pending_tr.pop(0)()
                drainq.append((g, sc, sl, accs))
        while drainq:
            pop_drain()

        # ================= Phase 3 chunk generator ==========================
        # P3 computes oT[d, q] = sum_h wo[hd, d] * at_h[hd, q], split into
        # three partials so most of it interleaves into the attention stream:
        # partial 0 = heads 0-3 (ready after block 15), partial 1 = heads 4,5
        # (ready after block 23), partial 2 = heads 6,7 (runs at the end).
        wo_tiles = []
        wo_dma_idx = [0]

        def wo_dma_next():
            i = wo_dma_idx[0]
            if i < 2 * c.DT:
                wt = wop.tile([128, 4 * c.HD], BF16, name="wo_t", tag="wo")
                nc.sync.dma_start(wt[:], wo_d[i // c.DT, i % c.DT])
                wo_tiles.append(wt)
                wo_dma_idx[0] += 1

        P3_HEADS = [(0, 1, 2, 3), (4, 5, 6, 7)]
        P3_OFF = [0, 0]  # column offset into the group wo tile

        def make_p3_part(p, endgame=False):
            """64 closures, dt-major; each computes one (dt, qc) output tile."""
            heads, off = P3_HEADS[p], P3_OFF[p]
            chunks = []
            for dt in range(c.DT):
                wt_ref = []

                def grab(wt_ref=wt_ref):
                    wt_ref.append(wo_tiles.pop(0))
                    wo_dma_next()

                for qc in range(c.NQC):
                    def chunk(p=p, dt=dt, qc=qc, wt_ref=wt_ref, grab=grab,
                              heads=heads, off=off):
                        if not wt_ref:
                            grab()
                        wt = wt_ref[0]
                        qsl = slice(qc * c.QCH, (qc + 1) * c.QCH)
                        if endgame and (dt + qc) % 2 == 1:
                            oT = p_ot(f"oT{p}{dt}{qc}")
                        else:
                            oT = p_zz(f"oT{p}{dt}{qc}")
                        for j, h8 in enumerate(heads):
                            nc.tensor.matmul(
                                oT[:],
                                wt[:, off + j * c.HD:off + (j + 1) * c.HD],
                                ats[h8][:, qsl],
                                start=(j == 0), stop=(j == len(heads) - 1),
                            )
                        oc = ocp.tile([128, c.QCH], BF16, name="oc", tag="oc")
                        if (dt + qc) % 2 == 0:
                            nc.scalar.copy(oc[:], oT[:])
                        else:
                            nc.vector.tensor_copy(oc[:], oT[:])
                        nc.sync.dma_start(
                            out_d[p, dt * 128:(dt + 1) * 128, qsl], oc[:]
                        )
                    chunks.append(chunk)
            return chunks

        # ================= Phase 2: causal attention ========================
        # Scores are computed in PAIRS: one [128,1024] PSUM tile = two k-tiles
        # for the same q-chunk; exp handles both in one Activation.
        for _ in range(3):
            wo_dma_next()
        p3_fifo = []

        prev_tail = None
        blocks = [(g, h, qc) for g in range(2) for h in range(4)
                  for qc in range(c.NQC)]
        for bi, (g, h, qc) in enumerate(blocks):
            # group 1's V transposes drip in during group 0's attention
            for _ in range(2):
                if pending_tr:
                    pending_tr.pop(0)()
            if bi == 16:
                p3_fifo.extend(make_p3_part(0))
            if p3_fifo and bi >= 17:
                for _ in range(2):
                    if p3_fifo:
                        p3_fifo.pop(0)()
            qh = g * 4 + h
            qsl = slice(qc * c.QCH, (qc + 1) * c.QCH)
            nkt = c.RB * (qc + 1)
            npair = nkt // 2
            ot = p_ot(f"ot{qh}{qc}")
            rsp = rp.tile([128, 2 * c.QCH], BF16, name=f"rs{qh}{qc}",
                          tag="rs", bufs=2)
            av_q = []
            for pk in range(npair):
                sp2 = p_stp2(f"sp{qh}{qc}{pk}")
                for half in range(2):
                    kt = 2 * pk + half
                    ridx = kt - (nkt - c.RB)
                    qlo = max(ridx, 0) * 128  # cols below the band are masked
                    o = half * c.QCH
                    nc.tensor.matmul(
                        sp2[:, o + qlo:o + c.QCH],
                        k_bf[g][:, kt * 128:(kt + 1) * 128],
                        q_bf[qh][:, qc * c.QCH + qlo:(qc + 1) * c.QCH],
                    )
                pt2 = ptp.tile([128, 2 * c.QCH], BF16, name="pt2", tag="pt2")
                nc.scalar.activation(pt2[:], sp2[:], AF.Exp, scale=scale)
                for half in range(2):
                    kt = 2 * pk + half
                    ridx = kt - (nkt - c.RB)
                    if ridx >= 0:  # diagonal band: causal mask
                        w = 128 * (ridx + 1)
                        o = half * c.QCH
                        nc.vector.tensor_tensor(
                            pt2[:, o:o + w], pt2[:, o:o + w],
                            cm_sb[:, ridx, 0:w], MUL
                        )
                if pk == 0:
                    nc.vector.tensor_copy(rsp[:], pt2[:])
                else:
                    nc.vector.tensor_tensor(rsp[:], rsp[:], pt2[:], ADD)
                def emit_av(k2, p2, nkt=nkt, g=g, ot=ot):
                    for half in range(2):
                        kt = 2 * k2 + half
                        ridx = kt - (nkt - c.RB)
                        qlo = max(ridx, 0) * 128
                        o = half * c.QCH
                        nc.tensor.matmul(
                            ot[:, qlo:], vn[g][:, kt, :],
                            p2[:, o + qlo:o + c.QCH],
                            start=(kt == 0), stop=(kt == nkt - 1),
                        )

                av_q.append((pk, pt2))
                if len(av_q) > 1:
                    emit_av(*av_q.pop(0))
                if pk == 1 and prev_tail is not None:
                    prev_tail()
                    prev_tail = None
                elif pk > 1 and pk % 2 == 1 and p3_fifo:
                    p3_fifo.pop(0)()
            for k2, p2 in av_q:
                emit_av(k2, p2)

            def make_tail(ot=ot, rsp=rsp, qh=qh, qsl=qsl, qc=qc):
                def tail():
                    zzt = p_zz(f"zz{qh}{qc}", [1, c.QCH])
                    nc.tensor.matmul(zzt[:], onec_sb[:], rsp[:, :c.QCH],
                                     start=True, stop=False)
                    nc.tensor.matmul(zzt[:], onec_sb[:], rsp[:, c.QCH:],
                                     start=False, stop=True)
                    zr = rp.tile([1, c.QCH], F32, name=f"zr{qh}{qc}",
                                 tag="zr", bufs=2)
                    nc.vector.reciprocal_approx_fast(zr[:], zzt[:])
                    zrb = rp.tile([1, c.QCH], BF16, name=f"zrb{qh}{qc}",
                                  tag="zrb", bufs=2)
                    nc.scalar.copy(zrb[:], zr[:])
                    zbp = p_zz(f"zbp{qh}{qc}")
                    nc.tensor.matmul(zbp[:], oner_sb[:], zrb[:])
                    zb = rp.tile([128, c.QCH], BF16, name=f"zb{qh}{qc}",
                                 tag="zb", bufs=2)
                    nc.vector.tensor_copy(zb[:], zbp[:])
                    nc.vector.tensor_tensor(ats[qh][:, qsl], ot[:], zb[:], MUL)
                return tail

            prev_tail = make_tail()
        prev_tail()

        # ================= Phase 3 remainder ================================
        for ch in p3_fifo:
            ch()
        for ch in make_p3_part(1, endgame=True):
            ch()

    nc.compile()
    nc.finalize()
    return nc


# ---------------------------------------------------------------------------
# Host-side sharding / gathering
# ---------------------------------------------------------------------------

def host_prep(x, freq_cis, wq, wk, wv, wo, n_cores, cfg: Cfg):
    import ml_dtypes
    BF = ml_dtypes.bfloat16
    c = cfg
    HD, HQC = c.HD, c.HQC

    x = np.asarray(x, np.float32)
    freq_cis = np.asarray(freq_cis, np.float32)
    wq = np.asarray(wq, np.float32)
    wk = np.asarray(wk, np.float32)
    wv = np.asarray(wv, np.float32)
    wo = np.asarray(wo, np.float32)

    # rope tables, interleaved layout: out[p] = ra[p]*t[p] + rb[p]*t[partner(p)]
    a = freq_cis[:, :, 0, 0].T
    bb = freq_cis[:, :, 0, 1].T
    cc = freq_cis[:, :, 1, 0].T
    dd = freq_cis[:, :, 1, 1].T
    S_ = freq_cis.shape[0]
    ra = np.empty((HD, S_), np.float32)
    rb = np.empty((HD, S_), np.float32)
    ra[0::2], ra[1::2] = a, dd
    rb[0::2], rb[1::2] = bb, cc

    pm = np.zeros((HD, HD), np.float32)
    idx = np.arange(HD)
    pm[idx, idx ^ 1] = 1.0

    # causal band masks: cm[k, m, q] = 1 if (k + 128*m) <= q
    ks = np.arange(128)[:, None]
    qs = np.arange(c.QCH)[None, :]
    cm = np.stack(
        [(ks + 128 * m <= qs) for m in range(c.RB)], axis=1
    ).astype(BF)

    in_maps = []
    for core in range(n_cores):
        b, hh = core // 2, core % 2
        hq0 = hh * HQC
        xt = np.ascontiguousarray(x[b].T.astype(BF))

        wq_c = wq[hq0 * HD:(hq0 + HQC) * HD]              # [1024, D]
        wq_p = np.ascontiguousarray(
            wq_c.T.reshape(c.DT, 128, 2, 4 * HD).transpose(2, 0, 1, 3).astype(BF)
        )
        wk_c = wk[2 * hh * HD:(2 * hh + 2) * HD]           # [256, D]
        wk_p = np.ascontiguousarray(wk_c.T.reshape(c.DT, 128, 2 * HD).astype(BF))
        wv_c = wv[2 * hh * HD:(2 * hh + 2) * HD]
        wv_p = np.ascontiguousarray(wv_c.T.reshape(c.DT, 128, 2 * HD).astype(BF))
        wo_c = wo[:, hq0 * HD:(hq0 + HQC) * HD]            # [D, 1024]
        # [2, DT, 128, 512]: per kv-group of 4 heads
        wo_p = np.ascontiguousarray(
            wo_c.T.reshape(2, 4, 128, c.DT, 128).transpose(0, 3, 2, 1, 4)
            .reshape(2, c.DT, 128, 4 * HD).astype(BF)
        )
        in_maps.append({
            "xt": xt,
            "wq": wq_p,
            "wk": wk_p,
            "wv": wv_p,
            "wo": wo_p,
            "ra": ra,
            "rb": rb,
            "cm": cm,
            "pm": pm,
            "idn": np.eye(128, dtype=BF),
            "onec": np.ones((HD, 1), BF),
            "oner": np.ones((1, HD), BF),
        })
    return in_maps


def run(inputs: dict, n_cores: int = 8, cfg: Cfg = Cfg(), trace: bool = False):
    in_maps = host_prep(
        inputs["x"], inputs["freq_cis"], inputs["wq"], inputs["wk"],
        inputs["wv"], inputs["wo"], n_cores, cfg,
    )
    nc = build_program(cfg)
    res = run_bass_kernel_spmd(nc, in_maps, list(range(n_cores)), trace=trace)
    out = np.empty((cfg.B, cfg.S, cfg.D), np.float32)
    for b in range(cfg.B):
        acc = np.zeros((cfg.D, cfg.S), np.float32)
        for core in (2 * b, 2 * b + 1):
            p = np.asarray(res.results[core]["partialT"]).astype(np.float32)
            acc += p[0] + p[1]
        out[b] = acc.T
    return out, res


def kernel(**inputs) -> np.ndarray:
    out, _ = run(inputs, n_cores=8, cfg=Cfg())
    return out


# revision 23
# speedup vs baseline: 1.0505x; 1.0505x over previous
"""Trainium2 Bass kernel for GQA attention (RoPE + causal) + output projection.

Sharding: (batch, head-half) across 8 cores. Core c handles batch c//2 and
q-heads [8*(c%2), 8*(c%2)+8) with kv-heads {2*(c%2), 2*(c%2)+1}. Each core
writes two transposed partial outputs [2, D, S] in bf16 (one per kv-group);
the host sums the four partials per batch and transposes back.

Engine plan (per core):
- PE: projections (bf16), rope pair-swap, scores/AV (bf16), softmax-denominator
  column sums + broadcasts, output projection (bf16). Emission is software-
  pipelined (drains overlapped into following matmul streams, paired-score
  lookahead, deferred softmax tails, group-0 output projection interleaved
  into group-1's attention) to keep the PE stream dense so it holds the
  2.4 GHz p-state.
- Scalar (Act): rope PSUM->SBUF copies, exp on score PAIRS ([128,1024] across
  two PSUM banks, halving instruction overhead), vn copies, half the P3 copies.
- DVE: rope swap-mult, causal mask mult (bf16 2x), softmax partial-sum adds
  (bf16 2x), reciprocal_approx_fast, denominator stage copy, final normalize,
  half the P3 copies.
- Pool (gpsimd): rope mult/add (SBUF-only; Pool cannot touch PSUM), bf16 casts
  of the reciprocal row.
"""

import math
from contextlib import ExitStack
from dataclasses import dataclass

import numpy as np

import concourse.bass as bass
import concourse.tile as tile
from concourse import bacc, mybir
from concourse.bass_utils import run_bass_kernel_spmd

F32 = mybir.dt.float32
F32R = mybir.dt.float32r
BF16 = mybir.dt.bfloat16
AF = mybir.ActivationFunctionType
MUL = mybir.AluOpType.mult
ADD = mybir.AluOpType.add


@dataclass(frozen=True)
class Cfg:
    B: int = 4          # batch
    S: int = 2048       # sequence length
    D: int = 2048       # model dim
    HQC: int = 8        # q-heads per core
    HD: int = 128       # head dim
    QCH: int = 512      # chunk (matmul moving free dim)

    @property
    def DT(self):
        return self.D // 128   # d-tiles

    @property
    def KT(self):
        return self.S // 128   # 128-row tiles along S

    @property
    def NQC(self):
        return self.S // self.QCH  # q-chunks

    @property
    def RB(self):
        return self.QCH // 128     # band tiles per q-chunk


def r(ap):
    """View an fp32 AP as float32r for full-rate PE matmuls."""
    return ap.bitcast(F32R)


def build_program(cfg: Cfg):
    c = cfg
    assert c.HD == 128 and c.HQC == 8 and c.RB == 4
    nc = bacc.Bacc("TRN2", target_bir_lowering=False, debug=False)

    xt_d = nc.dram_tensor("xt", [c.D, c.S], BF16, kind="ExternalInput")
    wq_d = nc.dram_tensor("wq", [2, c.DT, 128, 4 * c.HD], BF16, kind="ExternalInput")
    wk_d = nc.dram_tensor("wk", [c.DT, 128, 2 * c.HD], BF16, kind="ExternalInput")
    wv_d = nc.dram_tensor("wv", [c.DT, 128, 2 * c.HD], BF16, kind="ExternalInput")
    wo_d = nc.dram_tensor("wo", [2, c.DT, 128, 4 * c.HD], BF16, kind="ExternalInput")
    ra_d = nc.dram_tensor("ra", [c.HD, c.S], F32, kind="ExternalInput")
    rb_d = nc.dram_tensor("rb", [c.HD, c.S], F32, kind="ExternalInput")
    cm_d = nc.dram_tensor("cm", [128, c.RB, c.QCH], BF16, kind="ExternalInput")
    pm_d = nc.dram_tensor("pm", [128, 128], F32, kind="ExternalInput")
    idn_d = nc.dram_tensor("idn", [128, 128], BF16, kind="ExternalInput")
    onec_d = nc.dram_tensor("onec", [c.HD, 1], BF16, kind="ExternalInput")
    oner_d = nc.dram_tensor("oner", [1, c.HD], BF16, kind="ExternalInput")
    out_d = nc.dram_tensor("partialT", [2, c.D, c.S], BF16, kind="ExternalOutput")

    scale = 1.0 / math.sqrt(c.HD)

    with tile.TileContext(nc) as tc, ExitStack() as ctx:
        ctx.enter_context(nc.allow_low_precision("bf16 internals; tol 2e-2"))
        const = ctx.enter_context(tc.tile_pool(name="const", bufs=1))
        wp = ctx.enter_context(tc.tile_pool(name="wp", bufs=1))
        wop = ctx.enter_context(tc.tile_pool(name="wop", bufs=3))
        xp = ctx.enter_context(tc.tile_pool(name="xp", bufs=7))
        qkp = ctx.enter_context(tc.tile_pool(name="qkp", bufs=1))
        rtp = ctx.enter_context(tc.tile_pool(name="rtp", bufs=1))
        ptp = ctx.enter_context(tc.tile_pool(name="ptp", bufs=3))
        rp = ctx.enter_context(tc.tile_pool(name="rp", bufs=2))
        ocp = ctx.enter_context(tc.tile_pool(name="ocp", bufs=3))
        ps = ctx.enter_context(
            tc.tile_pool(name="ps", bufs=1, space=bass.MemorySpace.PSUM)
        )

        # PSUM tags (8 banks): stp2 [128,1024] x2 = 4 banks (score pairs, P1 Q
        # accumulators, softmax tails), ot [128,512] x2 = 2 banks (AV
        # accumulators, P1 K/V accumulators), zz [128,512] x2 = 2 banks (rope
        # pair-swap, V transposes, P3 output accumulators).
        def p_stp2(name):
            return ps.tile([128, 2 * c.QCH], F32, name=name, tag="stp2", bufs=2)

        def p_ot(name):
            return ps.tile([128, c.QCH], F32, name=name, tag="ot", bufs=2)

        def p_zz(name, shape=None, dtype=F32):
            return ps.tile(shape or [128, c.QCH], dtype, name=name, tag="zz",
                           bufs=2)

        # ---- small constants (big tables ra/rb/cm DMA'd later) ----
        ra_sb = const.tile([128, c.S], F32, name="ra_sb")
        rb_sb = const.tile([128, c.S], F32, name="rb_sb")
        cm_sb = const.tile([128, c.RB, c.QCH], BF16, name="cm_sb")
        pm_sb = const.tile([128, 128], F32R, name="pm_sb")
        nc.sync.dma_start(pm_sb[:], r(pm_d[:]))
        idn_sb = const.tile([128, 128], BF16, name="idn_sb")
        nc.sync.dma_start(idn_sb[:], idn_d[:])
        onec_sb = const.tile([128, 1], BF16, name="onec_sb")
        nc.sync.dma_start(onec_sb[:], onec_d[:])
        oner_sb = const.tile([1, 128], BF16, name="oner_sb")
        nc.sync.dma_start(oner_sb[:], oner_d[:])

        # ---- resident weights / activations ----
        wq_sb = [
            wp.tile([128, c.DT, 4 * c.HD], BF16, name=f"wq{g}", tag=f"wq{g}")
            for g in range(2)
        ]
        wk_sb = wp.tile([128, c.DT, 2 * c.HD], BF16, name="wk_sb")
        wv_sb = wp.tile([128, c.DT, 2 * c.HD], BF16, name="wv_sb")

        q_bf = [
            qkp.tile([128, c.S], BF16, name=f"q{h}", tag=f"q{h}") for h in range(8)
        ]
        k_bf = [
            qkp.tile([128, c.S], BF16, name=f"k{g}", tag=f"k{g}") for g in range(2)
        ]
        vt = [
            qkp.tile([128, c.S], BF16, name=f"vt{g}", tag=f"vt{g}") for g in range(2)
        ]
        vn = [
            qkp.tile([128, c.KT, c.HD], BF16, name=f"vn{g}", tag=f"vn{g}")
            for g in range(2)
        ]
        ats = [
            qkp.tile([128, c.S], BF16, name=f"at{h}", tag=f"at{h}") for h in range(8)
        ]

        # ================= Phase 1: projections (+rope, V transpose) ========
        # Drains for s-chunk sc are emitted at the TOP of the next chunk's
        # loop (before the PSUM accumulators are re-allocated) so the WAR
        # dependencies are visible to the tile framework. First-read copies
        # are spread across Scalar/DVE; the pair-swap matmuls then give the
        # PE immediate work while the copies drain.
        def emit_drains(g, sc, sl, accs):
            ts = []
            for i in range(4):  # q heads: scalar/scalar/dve/dve copies
                t = rtp.tile([128, c.QCH], F32R, name=f"t{g}{sc}{i}",
                             tag="rt", bufs=3)
                if i < 2:
                    nc.scalar.copy(t[:], accs[i])
                else:
                    nc.vector.tensor_copy(t[:], accs[i])
                ts.append(t)
            tk = rtp.tile([128, c.QCH], F32R, name=f"tk{g}{sc}", tag="rt", bufs=3)
            nc.scalar.copy(tk[:], accs[4])
            ts.append(tk)
            # Pool cannot read PSUM; V drain goes on DVE (casts to bf16)
            nc.vector.tensor_copy(vt[g][:, sl], accs[5])
            # rope the 5 copied tensors (4 Q + K)
            dsts = [q_bf[g * 4 + i] for i in range(4)] + [k_bf[g]]
            for i, (t, dst) in enumerate(zip(ts, dsts)):
                rps = p_zz(f"rps{g}{sc}{i}")
                nc.tensor.matmul(rps[:], pm_sb[:], t[:])
                sw = rtp.tile([128, c.QCH], F32, name=f"sw{g}{sc}{i}",
                              tag="sw", bufs=2)
                nc.vector.tensor_tensor(sw[:], rps[:], rb_sb[:, sl], MUL)
                tr = rtp.tile([128, c.QCH], F32, name=f"tr{g}{sc}{i}",
                              tag="tr", bufs=2)
                nc.gpsimd.tensor_tensor(tr[:], t[:], ra_sb[:, sl], MUL)
                nc.gpsimd.tensor_tensor(dst[:, sl], tr[:], sw[:], ADD)

        def make_transposes(g):
            out = []
            for st_i in range(c.KT):
                def tr_one(st_i=st_i, g=g):
                    tp = p_zz(f"tp{g}{st_i}", [128, 128], BF16)
                    nc.tensor.transpose(
                        tp[:], vt[g][:, st_i * 128:(st_i + 1) * 128], idn_sb[:]
                    )
                    nc.scalar.copy(vn[g][:, st_i, :], tp[:])
                out.append(tr_one)
            return out

        drainq = []            # deferred (g, sc, sl, accs), depth 2
        pending_tr = []        # deferred V-transpose closures

        def pop_drain():
            dg, dsc, dsl, daccs = drainq.pop(0)
            emit_drains(dg, dsc, dsl, daccs)
            if dsc == c.NQC - 1:  # group dg's V is complete
                pending_tr.extend(make_transposes(dg))

        for g in range(2):
            for sc in range(c.NQC):
                sl = slice(sc * c.QCH, (sc + 1) * c.QCH)
                if len(drainq) >= 1:
                    pop_drain()
                pa = p_stp2(f"pa{g}{sc}")
                pb = p_stp2(f"pb{g}{sc}")
                accs = [pa[:, :c.QCH], pa[:, c.QCH:], pb[:, :c.QCH],
                        pb[:, c.QCH:], p_ot(f"ak{g}{sc}")[:],
                        p_ot(f"av{g}{sc}")[:]]
                for dt in range(c.DT):
                    if g == 0 and sc == 0:
                        # quarter/half strided DMAs: few issues, incremental
                        # availability just ahead of the consuming matmuls
                        if dt == 0:
                            nc.sync.dma_start(
                                wq_sb[0][:, :4, :],
                                wq_d[0, :4].rearrange("t p h -> p t h"))
                        if dt % 4 == 0:
                            q4 = dt // 4 + 1
                            if q4 < 4:
                                ds_ = slice(4 * q4, 4 * q4 + 4)
                                nc.sync.dma_start(
                                    wq_sb[0][:, ds_, :],
                                    wq_d[0, ds_].rearrange("t p h -> p t h"))
                        if dt == 0:
                            nc.sync.dma_start(
                                wk_sb[:, :8, :],
                                wk_d[:8].rearrange("t p h -> p t h"))
                            nc.sync.dma_start(
                                wv_sb[:, :8, :],
                                wv_d[:8].rearrange("t p h -> p t h"))
                        elif dt == 4:
                            nc.sync.dma_start(
                                wk_sb[:, 8:, :],
                                wk_d[8:].rearrange("t p h -> p t h"))
                            nc.sync.dma_start(
                                wv_sb[:, 8:, :],
                                wv_d[8:].rearrange("t p h -> p t h"))
                    elif g == 0 and sc == 1 and dt % 4 == 2:
                        q4 = dt // 4
                        ds_ = slice(4 * q4, 4 * q4 + 4)
                        nc.sync.dma_start(
                            wq_sb[1][:, ds_, :],
                            wq_d[1, ds_].rearrange("t p h -> p t h"))
                    xt_t = xp.tile([128, c.QCH], BF16, name="xt_t", tag="xt")
                    nc.sync.dma_start(xt_t[:], xt_d[dt * 128:(dt + 1) * 128, sl])
                    st, sp = dt == 0, dt == c.DT - 1
                    for i in range(4):
                        nc.tensor.matmul(
                            accs[i],
                            wq_sb[g][:, dt, i * c.HD:(i + 1) * c.HD],
                            xt_t[:], start=st, stop=sp,
                        )
                    nc.tensor.matmul(
                        accs[4], wk_sb[:, dt, g * c.HD:(g + 1) * c.HD],
                        xt_t[:], start=st, stop=sp,
                    )
                    nc.tensor.matmul(
                        accs[5], wv_sb[:, dt, g * c.HD:(g + 1) * c.HD],
                        xt_t[:], start=st, stop=sp,
                    )
                    # big rope/mask tables ride in on the Pool DMA queue so
                    # they never contend with the SP queue's x-tile stream
                    if g == 0 and sc == 0 and dt == 8:
                        nc.gpsimd.dma_start(ra_sb[:], ra_d[:])
                        nc.gpsimd.dma_start(rb_sb[:], rb_d[:])
                        nc.gpsimd.dma_start(cm_sb[:], cm_d[:])
                    # interleave deferred V transposes of the previous group
                    if pending_tr and dt % 2 == 1:
                        pending_tr.pop(0)()
                drainq.append((g, sc, sl, accs))
        while drainq:
            pop_drain()

        # ================= Phase 3 chunk generator ==========================
        # P3 computes oT[d, q] = sum_h wo[hd, d] * at_h[hd, q], split into
        # three partials so most of it interleaves into the attention stream:
        # partial 0 = heads 0-3 (ready after block 15), partial 1 = heads 4,5
        # (ready after block 23), partial 2 = heads 6,7 (runs at the end).
        wo_tiles = []
        wo_dma_idx = [0]

        def wo_dma_next():
            i = wo_dma_idx[0]
            if i < 2 * c.DT:
                wt = wop.tile([128, 4 * c.HD], BF16, name="wo_t", tag="wo")
                nc.sync.dma_start(wt[:], wo_d[i // c.DT, i % c.DT])
                wo_tiles.append(wt)
                wo_dma_idx[0] += 1

        P3_HEADS = [(0, 1, 2, 3), (4, 5, 6, 7)]
        P3_OFF = [0, 0]  # column offset into the group wo tile

        def make_p3_part(p, endgame=False):
            """64 closures, dt-major; each computes one (dt, qc) output tile."""
            heads, off = P3_HEADS[p], P3_OFF[p]
            chunks = []
            for dt in range(c.DT):
                wt_ref = []

                def grab(wt_ref=wt_ref):
                    wt_ref.append(wo_tiles.pop(0))
                    wo_dma_next()

                for qc in range(c.NQC):
                    def chunk(p=p, dt=dt, qc=qc, wt_ref=wt_ref, grab=grab,
                              heads=heads, off=off):
                        if not wt_ref:
                            grab()
                        wt = wt_ref[0]
                        qsl = slice(qc * c.QCH, (qc + 1) * c.QCH)
                        if endgame and (dt + qc) % 2 == 1:
                            oT = p_ot(f"oT{p}{dt}{qc}")
                        else:
                            oT = p_zz(f"oT{p}{dt}{qc}")
                        for j, h8 in enumerate(heads):
                            nc.tensor.matmul(
                                oT[:],
                                wt[:, off + j * c.HD:off + (j + 1) * c.HD],
                                ats[h8][:, qsl],
                                start=(j == 0), stop=(j == len(heads) - 1),
                            )
                        oc = ocp.tile([128, c.QCH], BF16, name="oc", tag="oc")
                        if (dt + qc) % 2 == 0:
                            nc.scalar.copy(oc[:], oT[:])
                        else:
                            nc.vector.tensor_copy(oc[:], oT[:])
                        nc.sync.dma_start(
                            out_d[p, dt * 128:(dt + 1) * 128, qsl], oc[:]
                        )
                    chunks.append(chunk)
            return chunks

        # ================= Phase 2: causal attention ========================
        # Scores are computed in PAIRS: one [128,1024] PSUM tile = two k-tiles
        # for the same q-chunk; exp handles both in one Activation.
        for _ in range(3):
            wo_dma_next()
        p3_fifo = []

        prev_tail = None
        blocks = [(g, h, qc) for g in range(2) for h in range(4)
                  for qc in range(c.NQC)]
        for bi, (g, h, qc) in enumerate(blocks):
            # group 1's V transposes drip in during group 0's attention
            for _ in range(2):
                if pending_tr:
                    pending_tr.pop(0)()
            if bi == 16:
                p3_fifo.extend(make_p3_part(0))
            if p3_fifo and bi >= 17:
                for _ in range(2):
                    if p3_fifo:
                        p3_fifo.pop(0)()
            qh = g * 4 + h
            qsl = slice(qc * c.QCH, (qc + 1) * c.QCH)
            nkt = c.RB * (qc + 1)
            npair = nkt // 2
            ot = p_ot(f"ot{qh}{qc}")
            rsp = rp.tile([128, 2 * c.QCH], BF16, name=f"rs{qh}{qc}",
                          tag="rs", bufs=2)
            av_q = []
            for pk in range(npair):
                sp2 = p_stp2(f"sp{qh}{qc}{pk}")
                for half in range(2):
                    kt = 2 * pk + half
                    ridx = kt - (nkt - c.RB)
                    qlo = max(ridx, 0) * 128  # cols below the band are masked
                    o = half * c.QCH
                    nc.tensor.matmul(
                        sp2[:, o + qlo:o + c.QCH],
                        k_bf[g][:, kt * 128:(kt + 1) * 128],
                        q_bf[qh][:, qc * c.QCH + qlo:(qc + 1) * c.QCH],
                    )
                pt2 = ptp.tile([128, 2 * c.QCH], BF16, name="pt2", tag="pt2")
                nc.scalar.activation(pt2[:], sp2[:], AF.Exp, scale=scale)
                for half in range(2):
                    kt = 2 * pk + half
                    ridx = kt - (nkt - c.RB)
                    if ridx >= 0:  # diagonal band: causal mask
                        w = 128 * (ridx + 1)
                        o = half * c.QCH
                        nc.vector.tensor_tensor(
                            pt2[:, o:o + w], pt2[:, o:o + w],
                            cm_sb[:, ridx, 0:w], MUL
                        )
                if pk == 0:
                    nc.vector.tensor_copy(rsp[:], pt2[:])
                else:
                    nc.vector.tensor_tensor(rsp[:], rsp[:], pt2[:], ADD)
                def emit_av(k2, p2, nkt=nkt, g=g, ot=ot):
                    for half in range(2):
                        kt = 2 * k2 + half
                        ridx = kt - (nkt - c.RB)
                        qlo = max(ridx, 0) * 128
                        o = half * c.QCH
                        nc.tensor.matmul(
                            ot[:, qlo:], vn[g][:, kt, :],
                            p2[:, o + qlo:o + c.QCH],
                            start=(kt == 0), stop=(kt == nkt - 1),
                        )

                av_q.append((pk, pt2))
                if len(av_q) > 1:
                    emit_av(*av_q.pop(0))
                if pk == 1 and prev_tail is not None:
                    prev_tail()
                    prev_tail = None
                elif pk > 1 and pk % 2 == 1 and p3_fifo:
                    p3_fifo.pop(0)()
            for k2, p2 in av_q:
                emit_av(k2, p2)

            def make_tail(ot=ot, rsp=rsp, qh=qh, qsl=qsl, qc=qc):
                def tail():
                    zzt = p_zz(f"zz{qh}{qc}", [1, c.QCH])
                    nc.tensor.matmul(zzt[:], onec_sb[:], rsp[:, :c.QCH],
                                     start=True, stop=False)
                    nc.tensor.matmul(zzt[:], onec_sb[:], rsp[:, c.QCH:],
                                     start=False, stop=True)
                    zr = rp.tile([1, c.QCH], F32, name=f"zr{qh}{qc}",
                                 tag="zr", bufs=2)
                    nc.vector.reciprocal_approx_fast(zr[:], zzt[:])
                    zrb = rp.tile([1, c.QCH], BF16, name=f"zrb{qh}{qc}",
                                  tag="zrb", bufs=2)
                    nc.scalar.copy(zrb[:], zr[:])
                    zbp = p_zz(f"zbp{qh}{qc}")
                    nc.tensor.matmul(zbp[:], oner_sb[:], zrb[:])
                    zb = rp.tile([128, c.QCH], BF16, name=f"zb{qh}{qc}",
                                 tag="zb", bufs=2)
                    nc.vector.tensor_copy(zb[:], zbp[:])
                    nc.vector.tensor_tensor(ats[qh][:, qsl], ot[:], zb[:], MUL)
                return tail

            prev_tail = make_tail()
        prev_tail()

        # ================= Phase 3 remainder ================================
        for ch in p3_fifo:
            ch()
        for ch in make_p3_part(1, endgame=True):
            ch()

    nc.compile()
    nc.finalize()
    return nc


# ---------------------------------------------------------------------------
# Host-side sharding / gathering
# ---------------------------------------------------------------------------

def host_prep(x, freq_cis, wq, wk, wv, wo, n_cores, cfg: Cfg):
    import ml_dtypes
    BF = ml_dtypes.bfloat16
    c = cfg
    HD, HQC = c.HD, c.HQC

    x = np.asarray(x, np.float32)
    freq_cis = np.asarray(freq_cis, np.float32)
    wq = np.asarray(wq, np.float32)
    wk = np.asarray(wk, np.float32)
    wv = np.asarray(wv, np.float32)
    wo = np.asarray(wo, np.float32)

    # rope tables, interleaved layout: out[p] = ra[p]*t[p] + rb[p]*t[partner(p)]
    a = freq_cis[:, :, 0, 0].T
    bb = freq_cis[:, :, 0, 1].T
    cc = freq_cis[:, :, 1, 0].T
    dd = freq_cis[:, :, 1, 1].T
    S_ = freq_cis.shape[0]
    ra = np.empty((HD, S_), np.float32)
    rb = np.empty((HD, S_), np.float32)
    ra[0::2], ra[1::2] = a, dd
    rb[0::2], rb[1::2] = bb, cc

    pm = np.zeros((HD, HD), np.float32)
    idx = np.arange(HD)
    pm[idx, idx ^ 1] = 1.0

    # causal band masks: cm[k, m, q] = 1 if (k + 128*m) <= q
    ks = np.arange(128)[:, None]
    qs = np.arange(c.QCH)[None, :]
    cm = np.stack(
        [(ks + 128 * m <= qs) for m in range(c.RB)], axis=1
    ).astype(BF)

    in_maps = []
    for core in range(n_cores):
        b, hh = core // 2, core % 2
        hq0 = hh * HQC
        xt = np.ascontiguousarray(x[b].T.astype(BF))

        wq_c = wq[hq0 * HD:(hq0 + HQC) * HD]              # [1024, D]
        wq_p = np.ascontiguousarray(
            wq_c.T.reshape(c.DT, 128, 2, 4 * HD).transpose(2, 0, 1, 3).astype(BF)
        )
        wk_c = wk[2 * hh * HD:(2 * hh + 2) * HD]           # [256, D]
        wk_p = np.ascontiguousarray(wk_c.T.reshape(c.DT, 128, 2 * HD).astype(BF))
        wv_c = wv[2 * hh * HD:(2 * hh + 2) * HD]
        wv_p = np.ascontiguousarray(wv_c.T.reshape(c.DT, 128, 2 * HD).astype(BF))
        wo_c = wo[:, hq0 * HD:(hq0 + HQC) * HD]            # [D, 1024]
        # [2, DT, 128, 512]: per kv-group of 4 heads
        wo_p = np.ascontiguousarray(
            wo_c.T.reshape(2, 4, 128, c.DT, 128).transpose(0, 3, 2, 1, 4)
            .reshape(2, c.DT, 128, 4 * HD).astype(BF)
        )
        in_maps.append({
            "xt": xt,
            "wq": wq_p,
            "wk": wk_p,
            "wv": wv_p,
            "wo": wo_p,
            "ra": ra,
            "rb": rb,
            "cm": cm,
            "pm": pm,
            "idn": np.eye(128, dtype=BF),
            "onec": np.ones((HD, 1), BF),
            "oner": np.ones((1, HD), BF),
        })
    return in_maps


def run(inputs: dict, n_cores: int = 8, cfg: Cfg = Cfg(), trace: bool = False):
    in_maps = host_prep(
        inputs["x"], inputs["freq_cis"], inputs["wq"], inputs["wk"],
        inputs["wv"], inputs["wo"], n_cores, cfg,
    )
    nc = build_program(cfg)
    res = run_bass_kernel_spmd(nc, in_maps, list(range(n_cores)), trace=trace)
    out = np.empty((cfg.B, cfg.S, cfg.D), np.float32)
    for b in range(cfg.B):
        acc = np.zeros((cfg.D, cfg.S), np.float32)
        for core in (2 * b, 2 * b + 1):
            p = np.asarray(res.results[core]["partialT"]).astype(np.float32)
            acc += p[0] + p[1]
        out[b] = acc.T
    return out, res


def kernel(**inputs) -> np.ndarray:
    out, _ = run(inputs, n_cores=8, cfg=Cfg())
    return out


# revision 25
# speedup vs baseline: 1.0951x; 1.0424x over previous
"""Trainium2 Bass kernel for GQA attention (RoPE + causal) + output projection.

Sharding: (batch, head-half) across 8 cores. Core c handles batch c//2 and
q-heads [8*(c%2), 8*(c%2)+8) with kv-heads {2*(c%2), 2*(c%2)+1}. Each core
writes two transposed partial outputs [2, D, S] in bf16 (one per kv-group);
the host sums the four partials per batch and transposes back.

Engine plan (per core):
- PE: projections (bf16), rope pair-swap, scores/AV (bf16), softmax-denominator
  column sums + broadcasts, output projection (bf16). Emission is software-
  pipelined (drains overlapped into following matmul streams, paired-score
  lookahead, deferred softmax tails, group-0 output projection interleaved
  into group-1's attention) to keep the PE stream dense so it holds the
  2.4 GHz p-state.
- Scalar (Act): rope PSUM->SBUF copies, exp on score PAIRS ([128,1024] across
  two PSUM banks, halving instruction overhead), vn copies, half the P3 copies.
- DVE: rope swap-mult, causal mask mult (bf16 2x), softmax partial-sum adds
  (bf16 2x), reciprocal_approx_fast, denominator stage copy, final normalize,
  half the P3 copies.
- Pool (gpsimd): rope mult/add (SBUF-only; Pool cannot touch PSUM), bf16 casts
  of the reciprocal row.
"""

import math
from contextlib import ExitStack
from dataclasses import dataclass

import numpy as np

import concourse.bass as bass
import concourse.tile as tile
from concourse import bacc, mybir
from concourse.bass_utils import run_bass_kernel_spmd

F32 = mybir.dt.float32
F32R = mybir.dt.float32r
BF16 = mybir.dt.bfloat16
AF = mybir.ActivationFunctionType
MUL = mybir.AluOpType.mult
ADD = mybir.AluOpType.add


@dataclass(frozen=True)
class Cfg:
    B: int = 4          # batch
    S: int = 2048       # sequence length
    D: int = 2048       # model dim
    HQC: int = 8        # q-heads per core
    HD: int = 128       # head dim
    QCH: int = 512      # chunk (matmul moving free dim)

    @property
    def DT(self):
        return self.D // 128   # d-tiles

    @property
    def KT(self):
        return self.S // 128   # 128-row tiles along S

    @property
    def NQC(self):
        return self.S // self.QCH  # q-chunks

    @property
    def RB(self):
        return self.QCH // 128     # band tiles per q-chunk


def r(ap):
    """View an fp32 AP as float32r for full-rate PE matmuls."""
    return ap.bitcast(F32R)


def build_program(cfg: Cfg):
    c = cfg
    assert c.HD == 128 and c.HQC == 8 and c.RB == 4
    nc = bacc.Bacc("TRN2", target_bir_lowering=False, debug=False)

    xt_d = nc.dram_tensor("xt", [c.D, c.S], BF16, kind="ExternalInput")
    wq_d = nc.dram_tensor("wq", [2, c.DT, 128, 4 * c.HD], BF16, kind="ExternalInput")
    wk_d = nc.dram_tensor("wk", [c.DT, 128, 2 * c.HD], BF16, kind="ExternalInput")
    wv_d = nc.dram_tensor("wv", [c.DT, 128, 2 * c.HD], BF16, kind="ExternalInput")
    wo_d = nc.dram_tensor("wo", [2, c.DT, 128, 4 * c.HD], BF16, kind="ExternalInput")
    ra_d = nc.dram_tensor("ra", [c.HD, c.S], F32, kind="ExternalInput")
    rb_d = nc.dram_tensor("rb", [c.HD, c.S], F32, kind="ExternalInput")
    cm_d = nc.dram_tensor("cm", [128, c.RB, c.QCH], BF16, kind="ExternalInput")
    pm_d = nc.dram_tensor("pm", [128, 128], F32, kind="ExternalInput")
    idn_d = nc.dram_tensor("idn", [128, 128], BF16, kind="ExternalInput")
    onec_d = nc.dram_tensor("onec", [c.HD, 1], BF16, kind="ExternalInput")
    oner_d = nc.dram_tensor("oner", [1, c.HD], BF16, kind="ExternalInput")
    out_d = nc.dram_tensor("partialT", [2, c.D, c.S], BF16, kind="ExternalOutput")

    scale = 1.0 / math.sqrt(c.HD)

    with tile.TileContext(nc) as tc, ExitStack() as ctx:
        ctx.enter_context(nc.allow_low_precision("bf16 internals; tol 2e-2"))
        const = ctx.enter_context(tc.tile_pool(name="const", bufs=1))
        wp = ctx.enter_context(tc.tile_pool(name="wp", bufs=1))
        wop = ctx.enter_context(tc.tile_pool(name="wop", bufs=3))
        xp = ctx.enter_context(tc.tile_pool(name="xp", bufs=7))
        qkp = ctx.enter_context(tc.tile_pool(name="qkp", bufs=1))
        rtp = ctx.enter_context(tc.tile_pool(name="rtp", bufs=1))
        ptp = ctx.enter_context(tc.tile_pool(name="ptp", bufs=4))
        rp = ctx.enter_context(tc.tile_pool(name="rp", bufs=2))
        ocp = ctx.enter_context(tc.tile_pool(name="ocp", bufs=3))
        ps = ctx.enter_context(
            tc.tile_pool(name="ps", bufs=1, space=bass.MemorySpace.PSUM)
        )

        # PSUM tags (8 banks): stp2 [128,1024] x2 = 4 banks (score pairs, P1 Q
        # accumulators, softmax tails), ot [128,512] x2 = 2 banks (AV
        # accumulators, P1 K/V accumulators), zz [128,512] x2 = 2 banks (rope
        # pair-swap, V transposes, P3 output accumulators).
        def p_stp2(name):
            return ps.tile([128, 2 * c.QCH], F32, name=name, tag="stp2", bufs=2)

        def p_ot(name):
            return ps.tile([128, c.QCH], F32, name=name, tag="ot", bufs=2)

        def p_zz(name, shape=None, dtype=F32):
            return ps.tile(shape or [128, c.QCH], dtype, name=name, tag="zz",
                           bufs=2)

        # ---- small constants (big tables ra/rb/cm DMA'd later) ----
        ra_sb = const.tile([128, c.S], F32, name="ra_sb")
        rb_sb = const.tile([128, c.S], F32, name="rb_sb")
        cm_sb = const.tile([128, c.RB, c.QCH], BF16, name="cm_sb")
        pm_sb = const.tile([128, 128], F32R, name="pm_sb")
        nc.sync.dma_start(pm_sb[:], r(pm_d[:]))
        idn_sb = const.tile([128, 128], BF16, name="idn_sb")
        nc.sync.dma_start(idn_sb[:], idn_d[:])
        onec_sb = const.tile([128, 1], BF16, name="onec_sb")
        nc.sync.dma_start(onec_sb[:], onec_d[:])
        oner_sb = const.tile([1, 128], BF16, name="oner_sb")
        nc.sync.dma_start(oner_sb[:], oner_d[:])

        # ---- resident weights / activations ----
        wq_sb = [
            wp.tile([128, c.DT, 4 * c.HD], BF16, name=f"wq{g}", tag=f"wq{g}")
            for g in range(2)
        ]
        wk_sb = wp.tile([128, c.DT, 2 * c.HD], BF16, name="wk_sb")
        wv_sb = wp.tile([128, c.DT, 2 * c.HD], BF16, name="wv_sb")

        q_bf = [
            qkp.tile([128, c.S], BF16, name=f"q{h}", tag=f"q{h}") for h in range(8)
        ]
        k_bf = [
            qkp.tile([128, c.S], BF16, name=f"k{g}", tag=f"k{g}") for g in range(2)
        ]
        vt = [
            qkp.tile([128, c.S], BF16, name=f"vt{g}", tag=f"vt{g}") for g in range(2)
        ]
        vn = [
            qkp.tile([128, c.KT, c.HD], BF16, name=f"vn{g}", tag=f"vn{g}")
            for g in range(2)
        ]
        ats = [
            qkp.tile([128, c.S], BF16, name=f"at{h}", tag=f"at{h}") for h in range(8)
        ]

        # ================= Phase 1: projections (+rope, V transpose) ========
        # Drains for s-chunk sc are emitted at the TOP of the next chunk's
        # loop (before the PSUM accumulators are re-allocated) so the WAR
        # dependencies are visible to the tile framework. First-read copies
        # are spread across Scalar/DVE; the pair-swap matmuls then give the
        # PE immediate work while the copies drain.
        def emit_drains(g, sc, sl, accs):
            ts = []
            for i in range(4):  # q heads: scalar/scalar/dve/dve copies
                t = rtp.tile([128, c.QCH], F32R, name=f"t{g}{sc}{i}",
                             tag="rt", bufs=3)
                if i < 2:
                    nc.scalar.copy(t[:], accs[i])
                else:
                    nc.vector.tensor_copy(t[:], accs[i])
                ts.append(t)
            tk = rtp.tile([128, c.QCH], F32R, name=f"tk{g}{sc}", tag="rt", bufs=3)
            nc.scalar.copy(tk[:], accs[4])
            ts.append(tk)
            # Pool cannot read PSUM; V drain goes on DVE (casts to bf16)
            nc.vector.tensor_copy(vt[g][:, sl], accs[5])
            # rope the 5 copied tensors (4 Q + K)
            dsts = [q_bf[g * 4 + i] for i in range(4)] + [k_bf[g]]
            for i, (t, dst) in enumerate(zip(ts, dsts)):
                rps = p_zz(f"rps{g}{sc}{i}")
                nc.tensor.matmul(rps[:], pm_sb[:], t[:])
                sw = rtp.tile([128, c.QCH], F32, name=f"sw{g}{sc}{i}",
                              tag="sw", bufs=2)
                nc.vector.tensor_tensor(sw[:], rps[:], rb_sb[:, sl], MUL)
                tr = rtp.tile([128, c.QCH], F32, name=f"tr{g}{sc}{i}",
                              tag="tr", bufs=2)
                nc.gpsimd.tensor_tensor(tr[:], t[:], ra_sb[:, sl], MUL)
                nc.gpsimd.tensor_tensor(dst[:, sl], tr[:], sw[:], ADD)

        def make_transposes(g):
            out = []
            for st_i in range(c.KT):
                def tr_one(st_i=st_i, g=g):
                    tp = p_zz(f"tp{g}{st_i}", [128, 128], BF16)
                    nc.tensor.transpose(
                        tp[:], vt[g][:, st_i * 128:(st_i + 1) * 128], idn_sb[:]
                    )
                    nc.scalar.copy(vn[g][:, st_i, :], tp[:])
                out.append(tr_one)
            return out

        drainq = []            # deferred (g, sc, sl, accs), depth 2
        pending_tr = []        # deferred V-transpose closures

        def pop_drain():
            dg, dsc, dsl, daccs = drainq.pop(0)
            emit_drains(dg, dsc, dsl, daccs)
            if dsc == c.NQC - 1:  # group dg's V is complete
                pending_tr.extend(make_transposes(dg))

        for g in range(2):
            for sc in range(c.NQC):
                sl = slice(sc * c.QCH, (sc + 1) * c.QCH)
                if len(drainq) >= 1:
                    pop_drain()
                pa = p_stp2(f"pa{g}{sc}")
                pb = p_stp2(f"pb{g}{sc}")
                accs = [pa[:, :c.QCH], pa[:, c.QCH:], pb[:, :c.QCH],
                        pb[:, c.QCH:], p_ot(f"ak{g}{sc}")[:],
                        p_ot(f"av{g}{sc}")[:]]
                for dt in range(c.DT):
                    if g == 0 and sc == 0:
                        # quarter/half strided DMAs: few issues, incremental
                        # availability just ahead of the consuming matmuls
                        if dt == 0:
                            nc.sync.dma_start(
                                wq_sb[0][:, :4, :],
                                wq_d[0, :4].rearrange("t p h -> p t h"))
                        if dt % 4 == 0:
                            q4 = dt // 4 + 1
                            if q4 < 4:
                                ds_ = slice(4 * q4, 4 * q4 + 4)
                                nc.sync.dma_start(
                                    wq_sb[0][:, ds_, :],
                                    wq_d[0, ds_].rearrange("t p h -> p t h"))
                        if dt == 0:
                            nc.sync.dma_start(
                                wk_sb[:, :8, :],
                                wk_d[:8].rearrange("t p h -> p t h"))
                            nc.sync.dma_start(
                                wv_sb[:, :8, :],
                                wv_d[:8].rearrange("t p h -> p t h"))
                        elif dt == 4:
                            nc.sync.dma_start(
                                wk_sb[:, 8:, :],
                                wk_d[8:].rearrange("t p h -> p t h"))
                            nc.sync.dma_start(
                                wv_sb[:, 8:, :],
                                wv_d[8:].rearrange("t p h -> p t h"))
                    elif g == 0 and sc == 2 and dt % 4 == 2:
                        q4 = dt // 4
                        ds_ = slice(4 * q4, 4 * q4 + 4)
                        nc.sync.dma_start(
                            wq_sb[1][:, ds_, :],
                            wq_d[1, ds_].rearrange("t p h -> p t h"))
                    xt_t = xp.tile([128, c.QCH], BF16, name="xt_t", tag="xt")
                    nc.sync.dma_start(xt_t[:], xt_d[dt * 128:(dt + 1) * 128, sl])
                    st, sp = dt == 0, dt == c.DT - 1
                    for i in range(4):
                        nc.tensor.matmul(
                            accs[i],
                            wq_sb[g][:, dt, i * c.HD:(i + 1) * c.HD],
                            xt_t[:], start=st, stop=sp,
                        )
                    nc.tensor.matmul(
                        accs[4], wk_sb[:, dt, g * c.HD:(g + 1) * c.HD],
                        xt_t[:], start=st, stop=sp,
                    )
                    nc.tensor.matmul(
                        accs[5], wv_sb[:, dt, g * c.HD:(g + 1) * c.HD],
                        xt_t[:], start=st, stop=sp,
                    )
                    # big rope/mask tables ride in on the Pool DMA queue,
                    # after the startup window; only the (off-critical-path)
                    # rope DVE/Pool ops wait on them
                    if g == 0 and sc == 0 and dt == 15:
                        nc.gpsimd.dma_start(ra_sb[:], ra_d[:])
                        nc.gpsimd.dma_start(rb_sb[:], rb_d[:])
                        nc.gpsimd.dma_start(cm_sb[:], cm_d[:])
                    # interleave deferred V transposes of the previous group
                    if pending_tr and dt % 2 == 1:
                        pending_tr.pop(0)()
                drainq.append((g, sc, sl, accs))
        while drainq:
            pop_drain()

        # ================= Phase 3 chunk generator ==========================
        # P3 computes oT[d, q] = sum_h wo[hd, d] * at_h[hd, q], split into
        # three partials so most of it interleaves into the attention stream:
        # partial 0 = heads 0-3 (ready after block 15), partial 1 = heads 4,5
        # (ready after block 23), partial 2 = heads 6,7 (runs at the end).
        wo_tiles = []
        wo_dma_idx = [0]

        def wo_dma_next():
            i = wo_dma_idx[0]
            if i < 2 * c.DT:
                wt = wop.tile([128, 4 * c.HD], BF16, name="wo_t", tag="wo")
                nc.sync.dma_start(wt[:], wo_d[i // c.DT, i % c.DT])
                wo_tiles.append(wt)
                wo_dma_idx[0] += 1

        P3_HEADS = [(0, 1, 2, 3), (4, 5, 6, 7)]
        P3_OFF = [0, 0]  # column offset into the group wo tile

        def make_p3_part(p, endgame=False):
            """64 closures, dt-major; each computes one (dt, qc) output tile."""
            heads, off = P3_HEADS[p], P3_OFF[p]
            chunks = []
            idx_ctr = [0]
            for dt in range(c.DT):
                wt_ref = []

                def grab(wt_ref=wt_ref):
                    wt_ref.append(wo_tiles.pop(0))
                    wo_dma_next()

                for qc in range(c.NQC):
                    def chunk(p=p, dt=dt, qc=qc, wt_ref=wt_ref, grab=grab,
                              heads=heads, off=off):
                        if not wt_ref:
                            grab()
                        wt = wt_ref[0]
                        qsl = slice(qc * c.QCH, (qc + 1) * c.QCH)
                        idx = idx_ctr[0]
                        idx_ctr[0] += 1
                        if endgame and idx % 2 == 1:
                            oT = p_ot(f"oT{p}{dt}{qc}")
                        else:
                            oT = p_zz(f"oT{p}{dt}{qc}")
                        for j, h8 in enumerate(heads):
                            nc.tensor.matmul(
                                oT[:],
                                wt[:, off + j * c.HD:off + (j + 1) * c.HD],
                                ats[h8][:, qsl],
                                start=(j == 0), stop=(j == len(heads) - 1),
                            )
                        oc = ocp.tile([128, c.QCH], BF16, name="oc", tag="oc")
                        if (idx // 2) % 2 == 0:
                            nc.scalar.copy(oc[:], oT[:])
                        else:
                            nc.vector.tensor_copy(oc[:], oT[:])
                        nc.sync.dma_start(
                            out_d[p, dt * 128:(dt + 1) * 128, qsl], oc[:]
                        )
                    chunks.append(chunk)
            return chunks

        # ================= Phase 2: causal attention ========================
        # Scores are computed in PAIRS: one [128,1024] PSUM tile = two k-tiles
        # for the same q-chunk; exp handles both in one Activation.
        for _ in range(3):
            wo_dma_next()
        p3_fifo = []

        prev_tail = None
        blocks = [(g, h, qc) for g in range(2) for h in range(4)
                  for qc in range(c.NQC)]
        for bi, (g, h, qc) in enumerate(blocks):
            # group 1's V transposes drip in during group 0's attention
            for _ in range(2):
                if pending_tr:
                    pending_tr.pop(0)()
            if bi == 16:
                p3_fifo.extend(make_p3_part(0))
            if p3_fifo and bi >= 17:
                for _ in range(2):
                    if p3_fifo:
                        p3_fifo.pop(0)()
            qh = g * 4 + h
            qsl = slice(qc * c.QCH, (qc + 1) * c.QCH)
            nkt = c.RB * (qc + 1)
            npair = nkt // 2
            ot = p_ot(f"ot{qh}{qc}")
            rsp = rp.tile([128, 2 * c.QCH], BF16, name=f"rs{qh}{qc}",
                          tag="rs", bufs=2)
            av_q = []
            for pk in range(npair):
                sp2 = p_stp2(f"sp{qh}{qc}{pk}")
                for half in range(2):
                    kt = 2 * pk + half
                    ridx = kt - (nkt - c.RB)
                    qlo = max(ridx, 0) * 128  # cols below the band are masked
                    o = half * c.QCH
                    nc.tensor.matmul(
                        sp2[:, o + qlo:o + c.QCH],
                        k_bf[g][:, kt * 128:(kt + 1) * 128],
                        q_bf[qh][:, qc * c.QCH + qlo:(qc + 1) * c.QCH],
                    )
                pt2 = ptp.tile([128, 2 * c.QCH], BF16, name="pt2", tag="pt2")
                nc.scalar.activation(pt2[:], sp2[:], AF.Exp, scale=scale)
                for half in range(2):
                    kt = 2 * pk + half
                    ridx = kt - (nkt - c.RB)
                    if ridx >= 0:  # diagonal band: causal mask
                        w = 128 * (ridx + 1)
                        o = half * c.QCH
                        nc.vector.tensor_tensor(
                            pt2[:, o:o + w], pt2[:, o:o + w],
                            cm_sb[:, ridx, 0:w], MUL
                        )
                if pk == 0:
                    nc.vector.tensor_copy(rsp[:], pt2[:])
                else:
                    nc.vector.tensor_tensor(rsp[:], rsp[:], pt2[:], ADD)
                def emit_av(k2, p2, nkt=nkt, g=g, ot=ot):
                    for half in range(2):
                        kt = 2 * k2 + half
                        ridx = kt - (nkt - c.RB)
                        qlo = max(ridx, 0) * 128
                        o = half * c.QCH
                        nc.tensor.matmul(
                            ot[:, qlo:], vn[g][:, kt, :],
                            p2[:, o + qlo:o + c.QCH],
                            start=(kt == 0), stop=(kt == nkt - 1),
                        )

                av_q.append((pk, pt2))
                if len(av_q) > 2:
                    emit_av(*av_q.pop(0))
                if pk == 1 and prev_tail is not None:
                    prev_tail()
                    prev_tail = None
                elif pk > 1 and pk % 2 == 1 and p3_fifo:
                    p3_fifo.pop(0)()
            for k2, p2 in av_q:
                emit_av(k2, p2)

            def make_tail(ot=ot, rsp=rsp, qh=qh, qsl=qsl, qc=qc):
                def tail():
                    zzt = p_zz(f"zz{qh}{qc}", [1, c.QCH])
                    nc.tensor.matmul(zzt[:], onec_sb[:], rsp[:, :c.QCH],
                                     start=True, stop=False)
                    nc.tensor.matmul(zzt[:], onec_sb[:], rsp[:, c.QCH:],
                                     start=False, stop=True)
                    zr = rp.tile([1, c.QCH], F32, name=f"zr{qh}{qc}",
                                 tag="zr", bufs=2)
                    nc.vector.reciprocal_approx_fast(zr[:], zzt[:])
                    zrb = rp.tile([1, c.QCH], BF16, name=f"zrb{qh}{qc}",
                                  tag="zrb", bufs=2)
                    nc.scalar.copy(zrb[:], zr[:])
                    zbp = p_zz(f"zbp{qh}{qc}")
                    nc.tensor.matmul(zbp[:], oner_sb[:], zrb[:])
                    zb = rp.tile([128, c.QCH], BF16, name=f"zb{qh}{qc}",
                                 tag="zb", bufs=2)
                    nc.vector.tensor_copy(zb[:], zbp[:])
                    nc.vector.tensor_tensor(ats[qh][:, qsl], ot[:], zb[:], MUL)
                return tail

            prev_tail = make_tail()
        prev_tail()

        # ================= Phase 3 remainder ================================
        for ch in p3_fifo:
            ch()
        for ch in make_p3_part(1, endgame=True):
            ch()

    nc.compile()
    nc.finalize()
    return nc


# ---------------------------------------------------------------------------
# Host-side sharding / gathering
# ---------------------------------------------------------------------------

def host_prep(x, freq_cis, wq, wk, wv, wo, n_cores, cfg: Cfg):
    import ml_dtypes
    BF = ml_dtypes.bfloat16
    c = cfg
    HD, HQC = c.HD, c.HQC

    x = np.asarray(x, np.float32)
    freq_cis = np.asarray(freq_cis, np.float32)
    wq = np.asarray(wq, np.float32)
    wk = np.asarray(wk, np.float32)
    wv = np.asarray(wv, np.float32)
    wo = np.asarray(wo, np.float32)

    # rope tables, interleaved layout: out[p] = ra[p]*t[p] + rb[p]*t[partner(p)]
    a = freq_cis[:, :, 0, 0].T
    bb = freq_cis[:, :, 0, 1].T
    cc = freq_cis[:, :, 1, 0].T
    dd = freq_cis[:, :, 1, 1].T
    S_ = freq_cis.shape[0]
    ra = np.empty((HD, S_), np.float32)
    rb = np.empty((HD, S_), np.float32)
    ra[0::2], ra[1::2] = a, dd
    rb[0::2], rb[1::2] = bb, cc

    pm = np.zeros((HD, HD), np.float32)
    idx = np.arange(HD)
    pm[idx, idx ^ 1] = 1.0

    # causal band masks: cm[k, m, q] = 1 if (k + 128*m) <= q
    ks = np.arange(128)[:, None]
    qs = np.arange(c.QCH)[None, :]
    cm = np.stack(
        [(ks + 128 * m <= qs) for m in range(c.RB)], axis=1
    ).astype(BF)

    in_maps = []
    for core in range(n_cores):
        b, hh = core // 2, core % 2
        hq0 = hh * HQC
        xt = np.ascontiguousarray(x[b].T.astype(BF))

        wq_c = wq[hq0 * HD:(hq0 + HQC) * HD]              # [1024, D]
        wq_p = np.ascontiguousarray(
            wq_c.T.reshape(c.DT, 128, 2, 4 * HD).transpose(2, 0, 1, 3).astype(BF)
        )
        wk_c = wk[2 * hh * HD:(2 * hh + 2) * HD]           # [256, D]
        wk_p = np.ascontiguousarray(wk_c.T.reshape(c.DT, 128, 2 * HD).astype(BF))
        wv_c = wv[2 * hh * HD:(2 * hh + 2) * HD]
        wv_p = np.ascontiguousarray(wv_c.T.reshape(c.DT, 128, 2 * HD).astype(BF))
        wo_c = wo[:, hq0 * HD:(hq0 + HQC) * HD]            # [D, 1024]
        # [2, DT, 128, 512]: per kv-group of 4 heads
        wo_p = np.ascontiguousarray(
            wo_c.T.reshape(2, 4, 128, c.DT, 128).transpose(0, 3, 2, 1, 4)
            .reshape(2, c.DT, 128, 4 * HD).astype(BF)
        )
        in_maps.append({
            "xt": xt,
            "wq": wq_p,
            "wk": wk_p,
            "wv": wv_p,
            "wo": wo_p,
            "ra": ra,
            "rb": rb,
            "cm": cm,
            "pm": pm,
            "idn": np.eye(128, dtype=BF),
            "onec": np.ones((HD, 1), BF),
            "oner": np.ones((1, HD), BF),
        })
    return in_maps


def run(inputs: dict, n_cores: int = 8, cfg: Cfg = Cfg(), trace: bool = False):
    in_maps = host_prep(
        inputs["x"], inputs["freq_cis"], inputs["wq"], inputs["wk"],
        inputs["wv"], inputs["wo"], n_cores, cfg,
    )
    nc = build_program(cfg)
    res = run_bass_kernel_spmd(nc, in_maps, list(range(n_cores)), trace=trace)
    out = np.empty((cfg.B, cfg.S, cfg.D), np.float32)
    for b in range(cfg.B):
        acc = np.zeros((cfg.D, cfg.S), np.float32)
        for core in (2 * b, 2 * b + 1):
            p = np.asarray(res.results[core]["partialT"]).astype(np.float32)
            acc += p[0] + p[1]
        out[b] = acc.T
    return out, res


def kernel(**inputs) -> np.ndarray:
    out, _ = run(inputs, n_cores=8, cfg=Cfg())
    return out
